# revision 1
# baseline (speedup 1.0000x reference)
"""DSDM memory-update kernel for Trainium2, SPMD across 8 NeuronCores.

Computation (per reference):
    d_i   = ||A_i - q_a||_2                      (i over 2M rows)
    min_d = min_i d_i
    new_ge = ge + ema_temp*(min_d - ge)
    append = min_d >= 0.95*new_ge
    w_i   = softmax(-d)_i * ema   (zeroed when append)
    A'    = A + w*(q_a - A);  M' = M + w*(q_c - M)
    out   = concat([A', M'], axis=1)

Key identity: softmax(-d)_i = exp(-d_i)/sum_j exp(-d_j) needs no max-shift in
f32 here (d in ~[6, 17] so exp(-d) in [4e-8, 2e-3]). Each core computes local
(min, sum(exp(-d))) over its row shard, one tiny AllGather combines them, and
the whole branch + softmax scale folds into one scalar gamma = ema*(1-append)/Z
so the update is out = A*(1 - gamma*e) + (gamma*e)*q per row.
"""

import sys

sys.path.insert(0, "/opt/trn_rl_repo")

import numpy as np

import concourse.bass as bass
import concourse.bacc as bacc
import concourse.mybir as mybir
import concourse.tile as tile
from concourse import bass_isa
from concourse.bass_utils import run_bass_kernel_spmd

# ---- problem constants (hardcoded; kernel.py must be self-contained) ----
N_MEM = 2_000_000
D = 64
C = 10
OD = D + C  # 74
TIME_PERIOD = 100.0
COEF = 0.95
EMA = 2.0 / (TIME_PERIOD + 1.0)          # 0.019801980198019802
EMA_TEMP = 2.0 / (EMA + 1.0)             # ~1.9611650485436893

N_CORES = 8
P = 128


def make_cfg(n_real_rows, k_main=64, k_outer=32, n_cores=N_CORES):
    """Static per-core tiling config."""
    rows_per_tile = P * k_main
    n_pad_rows = -(-n_real_rows // P) * P  # round up to 128
    n_main = n_pad_rows // rows_per_tile
    rem = n_pad_rows - n_main * rows_per_tile
    k_tail = rem // P
    assert n_main * rows_per_tile + k_tail * P == n_pad_rows
    return dict(
        n_real=n_real_rows,
        n_pad=n_pad_rows,
        k_main=k_main,
        k_outer=min(k_outer, k_main),
        n_main=n_main,
        k_tail=k_tail,
        n_cores=n_cores,
        n_e_cols=n_main * k_main + k_tail,
    )


def dsdm_kernel_body(tc, A, M, QA, QC, GE, IDT, QBLK, QCBLK, OUT, cfg):
    """Emit the kernel IR. All-caps args are DRAM APs (per-core shard)."""
    nc = tc.nc
    f32 = mybir.dt.float32
    K = cfg["k_main"]
    KT = cfg["k_tail"]
    n_main = cfg["n_main"]
    n_pad = cfg["n_pad"]
    n_real = cfg["n_real"]
    NE = cfg["n_e_cols"]
    n_tiles = n_main + (1 if KT else 0)
    reps = cfg.get("reps", 1)
    X = mybir.AxisListType.X
    ALU = mybir.AluOpType
    ACT = mybir.ActivationFunctionType
    PSUM = bass.MemorySpace.PSUM

    rows_main = n_main * P * K
    A_t = A[0:rows_main, :].rearrange("(t p k) d -> t p (k d)", p=P, k=K)
    M_t = M[0:rows_main, :].rearrange("(t p k) d -> t p (k d)", p=P, k=K)
    O_t = OUT[0:rows_main, :].rearrange("(t p k) d -> t p (k d)", p=P, k=K)
    if KT:
        A_tail = A[rows_main:n_pad, :].rearrange("(p k) d -> p (k d)", k=KT)
        M_tail = M[rows_main:n_pad, :].rearrange("(p k) d -> p (k d)", k=KT)
        O_tail = OUT[rows_main:n_pad, :].rearrange("(p k) d -> p (k d)", k=KT)

    KO = cfg["k_outer"]
    with (
        tc.tile_pool(name="const", bufs=1) as cpool,
        tc.tile_pool(name="a_in", bufs=5) as apool,
        tc.tile_pool(name="m_in", bufs=2) as mpool,
        tc.tile_pool(name="ws", bufs=4) as wspool,
        tc.tile_pool(name="outp", bufs=3) as opool,
        tc.tile_pool(name="dsub", bufs=2) as dbpool,
        tc.tile_pool(name="psA", bufs=1, space=PSUM) as psA,
        tc.tile_pool(name="psM", bufs=2, space=PSUM) as psM,
        tc.tile_pool(name="psT", bufs=2, space=PSUM) as psT,
        tc.tile_pool(name="dram", bufs=2, space="DRAM") as drampool,
    ):
        # ---------------- setup: replicated query tiles, persistent bufs ----
        q_rep = cpool.tile([P, D], f32)         # one q copy; K-bcast via AP
        qc_rep = cpool.tile([P, C], f32)
        e_sb = cpool.tile([P, NE], f32)         # exp(-d) per row, persistent
        d2_sb = cpool.tile([P, NE], f32)        # dist^2 then dist, persistent
        scal = cpool.tile([P, 16], f32)         # scalar scratch (partition 0 mostly)
        gam_b = cpool.tile([P, 1], f32)
        ngam_b = cpool.tile([P, 1], f32)
        idt = cpool.tile([P, P], f32)           # identity for PE transpose
        n_grp = -(-K // KO)
        bf16 = mybir.dt.bfloat16
        # bf16 for the PE outer-product operands: fp32 PE runs at 1/4 rate and
        # w*q is a tiny correction term, so bf16 inputs are accuracy-safe.
        qblk = cpool.tile([n_grp * KO, KO * D], bf16)  # block-diag q, per group
        qcblk = cpool.tile([n_grp * KO, KO * C], bf16)

        # DMA-replicate q into [P, K*D]: read the same 64 floats for every
        # (partition, k). Stride-0 source AP on a DRAM read. Setup DMAs go on
        # the ACT HWDGE ring (nc.scalar) to keep the SP ring for bulk traffic.
        qa_src = QA.rearrange("(o d) -> o d", o=1)  # [1, 64]
        qc_src = QC.rearrange("(o d) -> o d", o=1)  # [1, 10]
        nc.scalar.dma_start(q_rep, qa_src.broadcast_to((P, D)))
        nc.scalar.dma_start(qc_rep, qc_src.broadcast_to((P, C)))
        ge_sb = cpool.tile([1, 1], f32)
        nc.scalar.dma_start(ge_sb, GE.rearrange("(o d) -> o d", o=1))
        nc.scalar.dma_start(idt, IDT)
        # block-diagonal q / qc for the PE outer-product trick, built host-side
        # (SWDGE cast f32 -> bf16 during the DMA)
        nc.gpsimd.dma_start(qblk, QBLK)
        nc.gpsimd.dma_start(qcblk, QCBLK)

        for _rep in range(reps):
            _dsdm_one_pass(
                tc, cfg, A_t, M_t, O_t,
                A_tail if KT else None, M_tail if KT else None,
                O_tail if KT else None,
                q_rep, qc_rep, e_sb, d2_sb, scal, gam_b, ngam_b, idt, qblk,
                qcblk, ge_sb, apool, mpool, wspool, opool, psA, psM, psT,
                drampool, cpool, dbpool,
            )


def _dsdm_one_pass(tc, cfg, A_t, M_t, O_t, A_tail, M_tail, O_tail, q_rep,
                   qc_rep, e_sb, d2_sb, scal, gam_b, ngam_b, idt, qblk, qcblk,
                   ge_sb, apool, mpool, wspool, opool, psA, psM, psT, drampool,
                   cpool, dbpool):
    KB_MAX = max(1, cfg["k_main"] - round(0.70 * cfg["k_main"]))
    nc = tc.nc
    f32 = mybir.dt.float32
    K = cfg["k_main"]
    KT = cfg["k_tail"]
    n_main = cfg["n_main"]
    NE = cfg["n_e_cols"]
    n_tiles = n_main + (1 if KT else 0)
    X = mybir.AxisListType.X
    ALU = mybir.AluOpType
    ACT = mybir.ActivationFunctionType

    if True:
        # ---------------- phase A: dist^2 per row ---------------------------
        col = 0
        for t in range(n_tiles):
            k = K if t < n_main else KT
            a_dram = A_t[t] if t < n_main else A_tail
            a = apool.tile([P, K * D], f32, tag="a_in")
            nc.sync.dma_start(a[:, : k * D], a_dram)
            # a - q split ~70% GPSIMD (in-place) / 30% DVE (separate buffer —
            # same-tile writes would falsely serialize the two engines)
            kp = min(k, max(1, round(0.70 * k)))
            qbv = q_rep.unsqueeze(1)
            a3p = a[:, : kp * D].rearrange("p (k d) -> p k d", k=kp)
            nc.gpsimd.tensor_sub(a3p, a3p, qbv.broadcast_to((P, kp, D)))
            nc.scalar.activation(a[:, : kp * D], a[:, : kp * D], ACT.Square)
            a3 = a.rearrange("p (k d) -> p k d", k=K)[:, :kp, :]
            nc.vector.tensor_reduce(d2_sb[:, col : col + kp], a3, axis=X,
                                    op=ALU.add)
            if kp < k:
                kb = k - kp
                db = dbpool.tile([P, KB_MAX * D], f32, tag="db")
                db3 = db[:, : kb * D].rearrange("p (k d) -> p k d", k=kb)
                ab3 = a[:, kp * D : k * D].rearrange("p (k d) -> p k d", k=kb)
                nc.vector.tensor_sub(db3, ab3, qbv.broadcast_to((P, kb, D)))
                nc.scalar.activation(db[:, : kb * D], db[:, : kb * D],
                                     ACT.Square)
                db3 = db[:, : kb * D].rearrange("p (k d) -> p k d", k=kb)
                nc.vector.tensor_reduce(d2_sb[:, col + kp : col + k], db3,
                                        axis=X, op=ALU.add)
            col += k

        # ---------------- batched d=sqrt(d2), e=exp(-d), stats --------------
        # (batching keeps the ACT function table from thrashing per tile)
        nc.scalar.activation(d2_sb[:, :NE], d2_sb[:, :NE], ACT.Sqrt)  # now dist
        sloc = cpool.tile([P, 1], f32)
        nc.scalar.activation(e_sb[:, :NE], d2_sb[:, :NE], ACT.Exp, scale=-1.0,
                             accum_out=sloc)
        mloc = cpool.tile([P, 1], f32)
        nc.vector.tensor_reduce(mloc, d2_sb[:, :NE], axis=X, op=ALU.min)

        # Pad rows were filled host-side with a large constant: d ~ 8e4 so
        # exp(-d) underflows to exactly 0 and the min is unaffected.

        nmloc = cpool.tile([P, 1], f32)
        nc.vector.tensor_scalar_mul(nmloc, mloc, -1.0)
        nm_all = cpool.tile([P, 1], f32)
        s_all = cpool.tile([P, 1], f32)
        nc.gpsimd.partition_all_reduce(nm_all, nmloc, channels=P,
                                       reduce_op=bass_isa.ReduceOp.max)
        nc.gpsimd.partition_all_reduce(s_all, sloc, channels=P,
                                       reduce_op=bass_isa.ReduceOp.add)
        pack = cpool.tile([1, 8], f32)
        nc.vector.tensor_scalar_mul(pack[0:1, 0:1], nm_all[0:1, 0:1], -1.0)
        nc.vector.tensor_copy(pack[0:1, 1:2], s_all[0:1, 0:1])
        nc.vector.memset(pack[0:1, 2:8], 0.0)

        # ---------------- collective: AllGather the 8 (min, sum) pairs ------
        n_cores = cfg["n_cores"]
        if cfg.get("use_collective", True):
            cin = drampool.tile([1, 8], f32)
            cout = drampool.tile([n_cores, 8], f32)
            nc.sync.dma_start(cin, pack)
            nc.gpsimd.collective_compute(
                "AllGather",
                ALU.bypass,
                replica_groups=[list(range(n_cores))],
                ins=[cin[:, :].opt()],
                outs=[cout[:, :].opt()],
            )
            g8 = cpool.tile([n_cores, 8], f32)
            nc.sync.dma_start(g8, cout)

            # global min over cores (negate + max), global Z (add)
            ng = cpool.tile([n_cores, 1], f32)
            nc.vector.tensor_scalar_mul(ng, g8[:, 0:1], -1.0)
            ng_all = cpool.tile([n_cores, 1], f32)
            z_all = cpool.tile([n_cores, 1], f32)
            nc.gpsimd.partition_all_reduce(ng_all, ng, channels=n_cores,
                                           reduce_op=bass_isa.ReduceOp.max)
            nc.gpsimd.partition_all_reduce(z_all, g8[:, 1:2], channels=n_cores,
                                           reduce_op=bass_isa.ReduceOp.add)
        else:
            ng_all, z_all = nm_all, s_all  # single-core: locals are global

        # ---------------- scalar math on partition 0 ------------------------
        s0 = scal  # [P, 16] scratch; use row 0 columns
        mstar = s0[0:1, 0:1]
        nc.vector.tensor_scalar_mul(mstar, ng_all[0:1, 0:1], -1.0)
        zrec = s0[0:1, 1:2]
        nc.vector.reciprocal(zrec, z_all[0:1, 0:1])
        # new_ge = (1-ema_temp)*ge + ema_temp*mstar
        t1 = s0[0:1, 2:3]
        nc.vector.tensor_scalar_mul(t1, mstar, float(EMA_TEMP))
        t2 = s0[0:1, 3:4]
        nc.vector.tensor_scalar_mul(t2, ge_sb, float(1.0 - EMA_TEMP))
        newge = s0[0:1, 4:5]
        nc.vector.tensor_add(newge, t1, t2)
        thr = s0[0:1, 5:6]
        nc.vector.tensor_scalar_mul(thr, newge, float(COEF))
        app = s0[0:1, 6:7]
        nc.vector.tensor_tensor(app, mstar, thr, op=ALU.is_ge)
        keep = s0[0:1, 7:8]
        nc.vector.tensor_scalar(keep, app, -1.0, 1.0, op0=ALU.mult, op1=ALU.add)
        gam1 = s0[0:1, 8:9]
        nc.vector.tensor_mul(gam1, keep, zrec)
        gam = s0[0:1, 9:10]
        nc.vector.tensor_scalar_mul(gam, gam1, float(EMA))
        nc.gpsimd.partition_broadcast(gam_b, gam, channels=P)
        nc.vector.tensor_scalar_mul(ngam_b, gam_b, -1.0)

        # ---------------- phase C: out = a*(1-w) + w*q ----------------------
        # w*q per tile is rank-1 per row-group: computed on the idle PE as
        # wT.T @ block_diag(q) into PSUM (in KO-row sub-groups to fit PSUM);
        # DVE then does mult + add only.
        KO = cfg["k_outer"]
        col = 0
        for t in range(n_tiles):
            k = K if t < n_main else KT
            a_dram = A_t[t] if t < n_main else A_tail
            m_dram = M_t[t] if t < n_main else M_tail
            o_dram = O_t[t] if t < n_main else O_tail
            a = apool.tile([P, K * D], f32, tag="a_in")
            nc.sync.dma_start(a[:, : k * D], a_dram)
            m = mpool.tile([P, K * C], f32, tag="m_in")
            nc.scalar.dma_start(m[:, : k * C], m_dram)
            e_ap = e_sb[:, col : col + k]
            ws = wspool.tile([P, 2 * K], f32, tag="ws")
            w_ap = ws[:, 0:k]
            s_ap = ws[:, K : K + k]
            nc.vector.tensor_scalar_mul(w_ap, e_ap, gam_b[:, 0:1])  # w = gam*e
            nc.vector.tensor_scalar(s_ap, e_ap, ngam_b[:, 0:1], 1.0,
                                    op0=ALU.mult, op1=ALU.add)       # s = 1-gam*e
            s_b = s_ap.to_broadcast((P, k, D))
            s_bc = s_ap.to_broadcast((P, k, C))

            # wT = w.T via PE transpose; bounce PSUM->SBUF on ACT with a
            # bf16 cast (bf16 operands make the outer-product matmuls 4x)
            wt_ps = psT.tile([K, P], f32, tag="wt")
            nc.tensor.transpose(wt_ps[:k, :], w_ap, idt)
            wt_sb = wspool.tile([K, P], mybir.dt.bfloat16, tag="wt_sb")
            nc.scalar.copy(wt_sb[:k, :], wt_ps[:k, :])

            o = opool.tile([P, K * OD], f32, tag="outp")
            o3 = o.rearrange("p (k d) -> p k d", k=K)
            oA = o3[:, :k, 0:D]
            oM = o3[:, :k, D:OD]
            a3 = a.rearrange("p (k d) -> p k d", k=K)[:, :k, :]
            m3 = m.rearrange("p (k d) -> p k d", k=K)[:, :k, :]

            nc.vector.tensor_tensor(oA, a3, s_b, op=ALU.mult)        # a*s
            nc.gpsimd.tensor_tensor(oM, m3, s_bc, op=ALU.mult)       # m*s (Pool)

            # per KO-sub-group: PA[p,(kk,d)] = w[p,ko+kk]*q[d]; oA += PA
            for ko in range(0, k, KO):
                ks = min(KO, k - ko)
                pa = psA.tile([P, KO * D], f32, tag="pa")
                for j in range(0, ks * D, 512):
                    je = min(j + 512, ks * D)
                    nc.tensor.matmul(pa[:, j:je], wt_sb[ko : ko + ks, :],
                                     qblk[ko : ko + ks, j:je], start=True,
                                     stop=True)
                pm = psM.tile([P, KO * C], f32, tag="pm")
                nc.tensor.matmul(pm[:, : ks * C], wt_sb[ko : ko + ks, :],
                                 qcblk[ko : ko + ks, : ks * C], start=True,
                                 stop=True)
                pa3 = pa.rearrange("p (k d) -> p k d", k=KO)[:, :ks, :]
                pm3 = pm.rearrange("p (k d) -> p k d", k=KO)[:, :ks, :]
                oAs = o3[:, ko : ko + ks, 0:D]
                oMs = o3[:, ko : ko + ks, D:OD]
                nc.vector.tensor_tensor(oAs, oAs, pa3, op=ALU.add)   # += w*q
                nc.vector.tensor_tensor(oMs, oMs, pm3, op=ALU.add)   # += w*qc

            nc.sync.dma_start(o_dram, o[:, : k * OD])
            col += k


_BUILD_CACHE = {}


def build_nc(cfg):
    key = tuple(sorted(cfg.items()))
    if key in _BUILD_CACHE:
        return _BUILD_CACHE[key]
    nc = bacc.Bacc("TRN2", target_bir_lowering=False, debug=False,
                   num_devices=cfg["n_cores"])
    f32 = mybir.dt.float32
    n_pad = cfg["n_pad"]
    A = nc.dram_tensor("A", [n_pad, D], f32, kind="ExternalInput").ap()
    M = nc.dram_tensor("M", [n_pad, C], f32, kind="ExternalInput").ap()
    QA = nc.dram_tensor("QA", [D], f32, kind="ExternalInput").ap()
    QC = nc.dram_tensor("QC", [C], f32, kind="ExternalInput").ap()
    GE = nc.dram_tensor("GE", [1], f32, kind="ExternalInput").ap()
    IDT = nc.dram_tensor("IDT", [P, P], f32, kind="ExternalInput").ap()
    k = cfg["k_outer"]
    g = -(-cfg["k_main"] // k)
    QBLK = nc.dram_tensor("QBLK", [g * k, k * D], f32, kind="ExternalInput").ap()
    QCBLK = nc.dram_tensor("QCBLK", [g * k, k * C], f32, kind="ExternalInput").ap()
    OUT = nc.dram_tensor("OUT", [n_pad, OD], f32, kind="ExternalOutput").ap()
    with tile.TileContext(nc) as tc:
        dsdm_kernel_body(tc, A, M, QA, QC, GE, IDT, QBLK, QCBLK, OUT, cfg)
    nc.compile()
    _BUILD_CACHE[key] = nc
    return nc


PAD_VALUE = 1.0e4  # pad rows -> dist ~8e4 -> exp underflows to 0; min unaffected


def make_aux_inputs(cfg, qa, qc):
    """Host-built constant inputs: identity + block-diagonal query matrices.

    The block-diagonal is replicated along partitions once per KO-sub-group so
    PE matmul operands share a base partition (lhsT = wT[ko:ko+ks])."""
    k = cfg["k_outer"]
    g = -(-cfg["k_main"] // k)
    qblk = np.zeros((k, k * D), np.float32)
    qcblk = np.zeros((k, k * C), np.float32)
    for kk in range(k):
        qblk[kk, kk * D : (kk + 1) * D] = qa
        qcblk[kk, kk * C : (kk + 1) * C] = qc
    return {
        "IDT": np.eye(P, dtype=np.float32),
        "QBLK": np.tile(qblk, (g, 1)),
        "QCBLK": np.tile(qcblk, (g, 1)),
    }


def _shard_pad(x, n_cores, n_real, n_pad):
    """Split rows across cores, pad each shard to n_pad with PAD_VALUE rows."""
    shards = []
    pad = n_pad - n_real
    for c in range(n_cores):
        s = x[c * n_real : (c + 1) * n_real]
        if pad:
            s = np.concatenate(
                [s, np.full((pad, s.shape[1]), PAD_VALUE, dtype=np.float32)], axis=0
            )
        shards.append(np.ascontiguousarray(s, dtype=np.float32))
    return shards


_WARMED = False


def _warm_devices(n_cores, tries=7, wait=45.0):
    """Touch every core with a trivial op before the real run.

    The axon terminal occasionally reports NRT_EXEC_UNIT_UNRECOVERABLE on the
    first use after another session exited uncleanly, and recovers on its own
    within a couple of minutes — retry cheap ops until the mesh is healthy."""
    global _WARMED
    if _WARMED:
        return
    import time as _time

    import jax
    import jax.numpy as jnp

    last = None
    for t in range(tries):
        try:
            for d in jax.devices()[:n_cores]:
                y = jax.device_put(np.zeros(4, np.float32), d)
                assert float(jnp.sum(y).block_until_ready()) == 0.0
            _WARMED = True
            return
        except Exception as e:  # noqa: BLE001 - retry any backend error
            last = e
            _time.sleep(wait)
    raise RuntimeError(f"NeuronCores unavailable after {tries} tries") from last


def kernel(A, M, query_address, query_content, global_error, _trace=False):
    A = np.asarray(A, dtype=np.float32)
    M = np.asarray(M, dtype=np.float32)
    qa = np.ascontiguousarray(np.asarray(query_address, dtype=np.float32))
    qc = np.ascontiguousarray(np.asarray(query_content, dtype=np.float32))
    ge = np.ascontiguousarray(np.asarray(global_error, dtype=np.float32))

    n_total = A.shape[0]
    n_cores = N_CORES
    assert n_total % n_cores == 0
    n_real = n_total // n_cores
    cfg = make_cfg(n_real)
    nc = build_nc(cfg)
    _warm_devices(n_cores)

    a_sh = _shard_pad(A, n_cores, n_real, cfg["n_pad"])
    m_sh = _shard_pad(M, n_cores, n_real, cfg["n_pad"])
    aux = make_aux_inputs(cfg, qa, qc)
    in_maps = [
        {"A": a_sh[c], "M": m_sh[c], "QA": qa, "QC": qc, "GE": ge, **aux}
        for c in range(n_cores)
    ]
    res = run_bass_kernel_spmd(nc, in_maps, core_ids=list(range(n_cores)),
                               trace=False)
    outs = [res.results[c]["OUT"][:n_real] for c in range(n_cores)]
    full = np.concatenate(outs, axis=0)
    if _trace:
        kernel.last_results = res
    return full


if __name__ == "__main__":
    # smoke test with random data (no reference comparison here)
    rng = np.random.default_rng(0)
    A = rng.standard_normal((N_MEM, D), dtype=np.float32)
    M = rng.standard_normal((N_MEM, C), dtype=np.float32)
    qa = rng.standard_normal(D).astype(np.float32)
    qc = rng.standard_normal(C).astype(np.float32)
    ge = rng.random(1, dtype=np.float32)
    out = kernel(A, M, qa, qc, ge)
    print("out", out.shape, out.dtype, float(np.abs(out).max()))

